# revision 11
# baseline (speedup 1.0000x reference)
"""Trainium2 Bass kernel for ChannelCompression:
   y = minmax_norm_spatial(leaky_relu(circulant_1x1_conv(x) + b))

Sharding: pure data parallel over batch (16 batches -> 2 per core x 8 cores).

Per-core strategy (memory-roofline bound: read x once, write y once):
  - View each batch as [C=16, G=8, S=32768] and stack (c,g) onto the 128
    SBUF partitions.  The circulant 16x16 conv becomes one 128x128
    block-structured matmul weight kron(W2.T, I8), so every PE column
    computes all 16 output channels for 8 spatial groups at once.
  - The matmul runs in float32r (single-pass relaxed fp32, 1 cycle/row
    for moving >= 256), so PE is never the bottleneck.
  - Pass 1 streams x tiles in, matmuls into PSUM (fp32), applies
    leaky-relu (+bias) on ScalarE while copying PSUM -> resident SBUF y
    buffer in bf16 (8 MiB/batch -> both batches fit, fully
    double-buffered).  Spatial min/max runs on DVE as elementwise
    running-min/max accumulator tiles (bf16 tensor_tensor, 2x_1p fast
    mode) -- tensor_reduce has no fast mode, so reducing every tile
    would make DVE the critical path.  The last tile fuses the
    accumulate with the final reduction via tensor_tensor_reduce.
  - Per-batch stats are folded across the 8 spatial groups via tiny PE
    transposes into free-dim space, reduced, inverted, and broadcast
    back to per-partition scale/bias with two tiny selector matmuls.
  - Pass 2 normalizes the resident bf16 y back to fp32 and streams it
    out.  In the middle phase (overlapped with pass 1 of the next batch)
    GpSimd does the normalize; in the tail phase the work round-robins
    over GpSimd / Vector / Scalar.  Output DMA kicks are issued from the
    Scalar queue with a 1-tile lag so the kick's semaphore wait never
    blocks the next PReLU issue.
"""

import numpy as np
from contextlib import ExitStack

import concourse.bacc as bacc
import concourse.tile as tile
import concourse.bass as bass
from concourse import mybir
from concourse.bass_utils import run_bass_kernel_spmd

F32 = mybir.dt.float32
F32R = mybir.dt.float32r
BF16 = mybir.dt.bfloat16
AF = mybir.ActivationFunctionType
ALU = mybir.AluOpType
AX = mybir.AxisListType

N_CORES = 8
B, C, H, W = 16, 16, 512, 512
G = 8                   # spatial groups stacked into partitions
BP = B // N_CORES       # batches per core
S_FULL = (H * W) // G   # 32768 spatial elems per group
TS = 2048               # columns per tile
PT = 1024               # columns per PSUM tile (2 banks)
MM = 512                # columns per matmul (1 PSUM bank)
EPS = 1e-8
NEG_SLOPE = 0.1
LAG = 1                 # out-DMA kick lag (tiles) on the scalar queue
BIG = 3.0e38


def build_nc(S=S_FULL, ts=TS):
    n_t = S // ts
    nc = bacc.Bacc("TRN2", target_bir_lowering=False)

    xs = nc.dram_tensor("x", [BP, C, G, S], F32R, kind="ExternalInput")
    wbd = nc.dram_tensor("wbd", [128, 128], F32R, kind="ExternalInput")
    ident = nc.dram_tensor("ident", [128, 128], F32, kind="ExternalInput")
    sel = nc.dram_tensor("sel", [32, 2, 128], F32, kind="ExternalInput")
    bb = nc.dram_tensor("b128", [128, 1], F32, kind="ExternalInput")
    ys = nc.dram_tensor("y", [BP, C, G, S], F32, kind="ExternalOutput")

    with tile.TileContext(nc) as tc, ExitStack() as ctx:
        consts = ctx.enter_context(tc.tile_pool(name="consts", bufs=1))
        xpool = ctx.enter_context(tc.tile_pool(name="xpool", bufs=5))
        ypool = ctx.enter_context(tc.tile_pool(name="ypool", bufs=2 * n_t))
        opool = ctx.enter_context(tc.tile_pool(name="opool", bufs=4))
        spool = ctx.enter_context(tc.tile_pool(name="stats", bufs=2))
        small = ctx.enter_context(tc.tile_pool(name="small", bufs=2))
        psum = ctx.enter_context(tc.tile_pool(name="psum", bufs=3, space="PSUM"))
        psmall = ctx.enter_context(tc.tile_pool(name="psmall", bufs=2, space="PSUM"))

        wbd_sb = consts.tile([128, 128], F32R)
        nc.sync.dma_start(out=wbd_sb, in_=wbd[:])
        id_sb = consts.tile([128, 128], F32)
        nc.sync.dma_start(out=id_sb, in_=ident[:])
        sel_sb = consts.tile([32, 2, 128], F32)
        nc.sync.dma_start(out=sel_sb, in_=sel[:])
        b_sb = consts.tile([128, 1], F32)
        nc.sync.dma_start(out=b_sb, in_=bb[:])
        # warm the Prelu activation table during the DMA ramp so the first
        # real PReLU doesn't pay the ACT_TABLE_LOAD latency
        warm = consts.tile([128, 1], F32)
        nc.scalar.activation(
            out=warm, in_=b_sb, func=AF.Prelu, bias=b_sb, scale=1.0,
            alpha=NEG_SLOPE,
        )

        state = {}
        pending_kicks = []  # (bi, i, ot) awaiting a lagged scalar dma kick

        def pass1_tile(bi, i):
            """DMA in x tile i of batch bi, conv+lrelu into resident bf16 y."""
            st_min, st_max, y_tiles = state[bi]
            xt = xpool.tile([128, ts], F32R, tag="x")
            nc.sync.dma_start(out=xt, in_=xs[bi, :, :, i * ts:(i + 1) * ts])
            yt = ypool.tile([128, ts], BF16, tag="y")
            for j in range(ts // PT):
                pt = psum.tile([128, PT], F32, tag="ps")
                for k in range(PT // MM):
                    c0 = k * MM
                    nc.tensor.matmul(
                        pt[:, c0:c0 + MM],
                        wbd_sb,
                        xt[:, j * PT + c0:j * PT + c0 + MM],
                        start=True,
                        stop=True,
                    )
                # y = leaky_relu(conv + b): fused PSUM->SBUF (bf16) on ScalarE
                nc.scalar.activation(
                    out=yt[:, j * PT:(j + 1) * PT],
                    in_=pt,
                    func=AF.Prelu,
                    bias=b_sb,
                    scale=1.0,
                    alpha=NEG_SLOPE,
                )
            # per-tile min/max stats on DVE (bf16 input reduce is fast on hw)
            nc.vector.tensor_reduce(
                out=st_min[:, i:i + 1], in_=yt, axis=AX.X, op=ALU.min
            )
            nc.vector.tensor_reduce(
                out=st_max[:, i:i + 1], in_=yt, axis=AX.X, op=ALU.max
            )
            y_tiles.append(yt)

        def stats_fold(bi):
            """Fold per-partition stats into per-partition scale/bias [128,2]."""
            st_min, st_max = state[bi][:2]
            s2 = small.tile([128, 2], F32, tag="s2")
            nc.vector.tensor_reduce(out=s2[:, 0:1], in_=st_min, axis=AX.X, op=ALU.min)
            nc.vector.tensor_reduce(out=s2[:, 1:2], in_=st_max, axis=AX.X, op=ALU.max)
            # transpose [128,1] stats into free dim (partition 0)
            ptr_min = psmall.tile([1, 128], F32, tag="psm")
            nc.tensor.transpose(ptr_min, s2[:, 0:1], id_sb)
            ptr_max = psmall.tile([1, 128], F32, tag="psm")
            nc.tensor.transpose(ptr_max, s2[:, 1:2], id_sb)
            tl = small.tile([1, 256], F32, tag="tl")
            nc.scalar.copy(out=tl[:, 0:128], in_=ptr_min)
            nc.scalar.copy(out=tl[:, 128:256], in_=ptr_max)
            # reduce over the 8 groups (free index p = o*8+g)
            u = small.tile([1, 32], F32, tag="u")
            nc.vector.tensor_reduce(
                out=u[:, 0:16],
                in_=tl[:, 0:128].rearrange("p (o g) -> p o g", g=G),
                axis=AX.X,
                op=ALU.min,
            )
            nc.vector.tensor_reduce(
                out=u[:, 16:32],
                in_=tl[:, 128:256].rearrange("p (o g) -> p o g", g=G),
                axis=AX.X,
                op=ALU.max,
            )
            # scale = 1/(mx-mn+eps); nbias = -mn*scale
            v = small.tile([1, 16], F32, tag="v")
            nc.vector.tensor_sub(out=v, in0=u[:, 16:32], in1=u[:, 0:16])
            vv = small.tile([1, 16], F32, tag="vv")
            nc.vector.tensor_scalar(
                out=vv, in0=v, scalar1=EPS, scalar2=None, op0=ALU.add
            )
            pk = small.tile([1, 32], F32, tag="pk")
            nc.vector.reciprocal(out=pk[:, 0:16], in_=vv)
            tmp = small.tile([1, 16], F32, tag="tmp")
            nc.vector.tensor_mul(out=tmp, in0=u[:, 0:16], in1=pk[:, 0:16])
            nc.vector.tensor_scalar(
                out=pk[:, 16:32], in0=tmp, scalar1=-1.0, scalar2=None, op0=ALU.mult
            )
            # broadcast [1,32] free-dim -> per-partition [128,2] via transpose
            # + selector matmuls (sel[k,0,p]=d(k==p//8), sel[k,1,p]=d(k-16==p//8))
            pz = psmall.tile([32, 1], F32, tag="psm")
            nc.tensor.transpose(pz, pk, id_sb[0:1, 0:1])
            zs = small.tile([32, 1], F32, tag="zs")
            nc.scalar.copy(out=zs, in_=pz)
            pb = psmall.tile([128, 2], F32, tag="psm")
            nc.tensor.matmul(pb[:, 0:1], sel_sb[:, 0, :], zs, start=True, stop=True)
            nc.tensor.matmul(pb[:, 1:2], sel_sb[:, 1, :], zs, start=True, stop=True)
            sc = small.tile([128, 2], F32, tag="sc", name=f"sc{bi}")
            nc.scalar.copy(out=sc, in_=pb)
            return sc

        def pass2_norm(bi, i, sc, eng):
            """Normalize resident bf16 y tile -> fp32 out tile on `eng`."""
            y_tiles = state[bi][2]
            ot = opool.tile([128, ts], F32, tag="o")
            if eng == "scalar":
                # Prelu with alpha=1 is identity: out = y*scale + nbias
                nc.scalar.activation(
                    out=ot,
                    in_=y_tiles[i],
                    func=AF.Prelu,
                    bias=sc[:, 1:2],
                    scale=sc[:, 0:1],
                    alpha=1.0,
                )
            else:
                e = nc.gpsimd if eng == "gpsimd" else nc.vector
                e.tensor_scalar(
                    out=ot,
                    in0=y_tiles[i],
                    scalar1=sc[:, 0:1],
                    scalar2=sc[:, 1:2],
                    op0=ALU.mult,
                    op1=ALU.add,
                )
            pending_kicks.append((bi, i, ot))

        def flush_kicks(keep=0):
            """Issue queued out-DMA kicks (oldest first) on the scalar queue."""
            while len(pending_kicks) > keep:
                bi, i, ot = pending_kicks.pop(0)
                nc.scalar.dma_start(
                    out=ys[bi, :, :, i * ts:(i + 1) * ts], in_=ot
                )

        for bi in range(BP):
            state[bi] = (
                spool.tile([128, n_t], F32, tag="stmin", name=f"stmin{bi}"),
                spool.tile([128, n_t], F32, tag="stmax", name=f"stmax{bi}"),
                [],
            )

        # head: pass 1 of batch 0, fold immediately (PE/Scalar have slack
        # to absorb the fold ops sitting ahead of batch-1 work)
        for i in range(n_t):
            pass1_tile(0, i)
        sc0 = stats_fold(0)
        # middle: pass 2 of batch 0 (GpSimd) interleaved with pass 1 of batch 1
        for i in range(n_t):
            pass1_tile(1, i)
            pass2_norm(0, i, sc0, "gpsimd")
            flush_kicks(keep=LAG)
        sc1 = stats_fold(1)
        # tail: pass 2 of batch 1, round-robin over the idle engines
        engs = ("gpsimd", "vector", "scalar")
        for i in range(n_t):
            pass2_norm(1, i, sc1, engs[i % 3])
            flush_kicks(keep=LAG)
        flush_kicks(keep=0)

    nc.compile()
    return nc


def host_consts(w, b):
    """Host-side tiny constant tensors fed to every core."""
    w = np.asarray(w, np.float32).reshape(16)
    b = np.asarray(b, np.float32).reshape(1)
    W2 = np.stack([np.roll(w, o) for o in range(16)], axis=0)   # [O,C]
    wbd = np.kron(W2.T.copy(), np.eye(G, dtype=np.float32))     # [128,128]
    wbd = np.ascontiguousarray(wbd, np.float32)
    ident = np.eye(128, dtype=np.float32)
    sel = np.zeros((32, 2, 128), np.float32)
    for p in range(128):
        sel[p // G, 0, p] = 1.0
        sel[16 + p // G, 1, p] = 1.0
    b128 = np.full((128, 1), float(b[0]), np.float32)
    return wbd, ident, sel, b128


_NC = None
LAST_RESULTS = None


def kernel(x, w, b):
    global _NC, LAST_RESULTS
    x = np.ascontiguousarray(np.asarray(x, np.float32))
    assert x.shape == (B, C, H, W)
    if _NC is None:
        _NC = build_nc()
    wbd, ident, sel, b128 = host_consts(w, b)

    xg = x.reshape(N_CORES, BP, C, G, S_FULL)
    in_maps = [
        {
            "x": np.ascontiguousarray(xg[ci]),
            "wbd": wbd,
            "ident": ident,
            "sel": sel,
            "b128": b128,
        }
        for ci in range(N_CORES)
    ]
    res = run_bass_kernel_spmd(_NC, in_maps, core_ids=list(range(N_CORES)))
    LAST_RESULTS = res
    out = np.concatenate([r["y"].reshape(BP, C, H, W) for r in res.results], axis=0)
    return out


# revision 18
# speedup vs baseline: 1.1206x; 1.1206x over previous
"""Trainium2 Bass kernel for ChannelCompression:
   y = minmax_norm_spatial(leaky_relu(circulant_1x1_conv(x) + b))

Sharding: pure data parallel over batch (16 batches -> 2 per core x 8 cores).

Per-core strategy (memory-roofline bound: read x once, write y once):
  - View each batch as [C=16, G=8, S=32768] and stack (c,g) onto the 128
    SBUF partitions.  The circulant 16x16 conv becomes one 128x128
    block-structured matmul weight kron(W2.T, I8), so every PE column
    computes all 16 output channels for 8 spatial groups at once.
  - The matmul runs in float32r (single-pass relaxed fp32, 1 cycle/row
    for moving >= 256), so PE is never the bottleneck.
  - Pass 1 streams x tiles in, matmuls into PSUM (fp32), applies
    leaky-relu (+bias) on ScalarE while copying PSUM -> resident SBUF y
    buffer in bf16 (8 MiB/batch -> both batches fit, fully
    double-buffered).  Spatial min/max runs on DVE as elementwise
    running-min/max accumulator tiles (bf16 tensor_tensor, 2x_1p fast
    mode) -- tensor_reduce has no fast mode, so reducing every tile
    would make DVE the critical path.  The last tile fuses the
    accumulate with the final reduction via tensor_tensor_reduce.
  - Per-batch stats are folded across the 8 spatial groups via tiny PE
    transposes into free-dim space, reduced, inverted, and broadcast
    back to per-partition scale/bias with two tiny selector matmuls.
  - Pass 2 normalizes the resident bf16 y back to fp32 and streams it
    out.  In the middle phase (overlapped with pass 1 of the next batch)
    GpSimd does the normalize; in the tail phase the work round-robins
    over GpSimd / Vector / Scalar.  Output DMA kicks are issued from the
    Scalar queue with a 1-tile lag so the kick's semaphore wait never
    blocks the next PReLU issue.
"""

import numpy as np
from contextlib import ExitStack

import concourse.bacc as bacc
import concourse.tile as tile
import concourse.bass as bass
from concourse import mybir
from concourse.bass_utils import run_bass_kernel_spmd

F32 = mybir.dt.float32
F32R = mybir.dt.float32r
BF16 = mybir.dt.bfloat16
AF = mybir.ActivationFunctionType
ALU = mybir.AluOpType
AX = mybir.AxisListType

N_CORES = 8
B, C, H, W = 16, 16, 512, 512
G = 8                   # spatial groups stacked into partitions
BP = B // N_CORES       # batches per core
S_FULL = (H * W) // G   # 32768 spatial elems per group
TS = 2048               # columns per tile
PT = 1024               # columns per PSUM tile (2 banks)
MM = 512                # columns per matmul (1 PSUM bank)
EPS = 1e-8
NEG_SLOPE = 0.1
LAG = 1                 # out-DMA kick lag (tiles) on the scalar queue
BIG = 3.0e38


def build_nc(S=S_FULL, ts=TS):
    n_t = S // ts
    nc = bacc.Bacc("TRN2", target_bir_lowering=False)

    xs = nc.dram_tensor("x", [BP, C, G, S], F32R, kind="ExternalInput")
    wbd = nc.dram_tensor("wbd", [128, 128], F32R, kind="ExternalInput")
    ident = nc.dram_tensor("ident", [128, 128], F32, kind="ExternalInput")
    sel = nc.dram_tensor("sel", [32, 2, 128], F32, kind="ExternalInput")
    bb = nc.dram_tensor("b128", [128, 1], F32, kind="ExternalInput")
    ys = nc.dram_tensor("y", [BP, C, G, S], F32, kind="ExternalOutput")

    with tile.TileContext(nc) as tc, ExitStack() as ctx:
        consts = ctx.enter_context(tc.tile_pool(name="consts", bufs=1))
        xpool = ctx.enter_context(tc.tile_pool(name="xpool", bufs=4))
        ypool = ctx.enter_context(tc.tile_pool(name="ypool", bufs=2 * n_t))
        opool = ctx.enter_context(tc.tile_pool(name="opool", bufs=3))
        spool = ctx.enter_context(tc.tile_pool(name="stats", bufs=2))
        small = ctx.enter_context(tc.tile_pool(name="small", bufs=2))
        psum = ctx.enter_context(tc.tile_pool(name="psum", bufs=3, space="PSUM"))
        psmall = ctx.enter_context(tc.tile_pool(name="psmall", bufs=2, space="PSUM"))

        wbd_sb = consts.tile([128, 128], F32R)
        nc.sync.dma_start(out=wbd_sb, in_=wbd[:])
        id_sb = consts.tile([128, 128], F32)
        nc.sync.dma_start(out=id_sb, in_=ident[:])
        sel_sb = consts.tile([32, 2, 128], F32)
        nc.sync.dma_start(out=sel_sb, in_=sel[:])
        b_sb = consts.tile([128, 1], F32)
        nc.sync.dma_start(out=b_sb, in_=bb[:])
        # warm the Prelu activation table during the DMA ramp so the first
        # real PReLU doesn't pay the ACT_TABLE_LOAD latency
        warm = consts.tile([128, 1], F32)
        nc.scalar.activation(
            out=warm, in_=b_sb, func=AF.Prelu, bias=b_sb, scale=1.0,
            alpha=NEG_SLOPE,
        )

        state = {}
        pending_kicks = []  # (bi, i, ot) awaiting a lagged scalar dma kick

        def pass1_tile(bi, i):
            """DMA in x tile i of batch bi, conv+lrelu into resident bf16 y."""
            rmin, rmax, y_tiles = state[bi]
            xt = xpool.tile([128, ts], F32R, tag="x")
            nc.sync.dma_start(out=xt, in_=xs[bi, :, :, i * ts:(i + 1) * ts])
            yt = ypool.tile([128, ts], BF16, tag="y")
            for j in range(ts // PT):
                pt = psum.tile([128, PT], F32, tag="ps")
                for k in range(PT // MM):
                    c0 = k * MM
                    nc.tensor.matmul(
                        pt[:, c0:c0 + MM],
                        wbd_sb,
                        xt[:, j * PT + c0:j * PT + c0 + MM],
                        start=True,
                        stop=True,
                    )
                # y = leaky_relu(conv + b): fused PSUM->SBUF (bf16) on ScalarE
                nc.scalar.activation(
                    out=yt[:, j * PT:(j + 1) * PT],
                    in_=pt,
                    func=AF.Prelu,
                    bias=b_sb,
                    scale=1.0,
                    alpha=NEG_SLOPE,
                )
            # running elementwise min/max accumulators on DVE (bf16 ops get
            # the 2x fast mode there; tensor_reduce per tile has none and
            # would make DVE the pacer)
            if i == 0:
                nc.vector.tensor_scalar(
                    out=rmin, in0=yt, scalar1=BIG, scalar2=None, op0=ALU.min
                )
                nc.vector.tensor_scalar(
                    out=rmax, in0=yt, scalar1=-BIG, scalar2=None, op0=ALU.max
                )
            else:
                nc.vector.tensor_tensor(out=rmin, in0=rmin, in1=yt, op=ALU.min)
                nc.vector.tensor_tensor(out=rmax, in0=rmax, in1=yt, op=ALU.max)
            y_tiles.append(yt)

        def stats_fold(bi):
            """Fold per-partition stats into per-partition scale/bias [128,2]."""
            rmin, rmax = state[bi][:2]
            s2 = small.tile([128, 2], F32, tag="s2")
            nc.vector.tensor_reduce(out=s2[:, 0:1], in_=rmin, axis=AX.X, op=ALU.min)
            nc.vector.tensor_reduce(out=s2[:, 1:2], in_=rmax, axis=AX.X, op=ALU.max)
            # transpose [128,1] stats into free dim (partition 0)
            ptr_min = psmall.tile([1, 128], F32, tag="psm")
            nc.tensor.transpose(ptr_min, s2[:, 0:1], id_sb)
            ptr_max = psmall.tile([1, 128], F32, tag="psm")
            nc.tensor.transpose(ptr_max, s2[:, 1:2], id_sb)
            tl = small.tile([1, 256], F32, tag="tl")
            nc.scalar.copy(out=tl[:, 0:128], in_=ptr_min)
            nc.scalar.copy(out=tl[:, 128:256], in_=ptr_max)
            # reduce over the 8 groups (free index p = o*8+g)
            u = small.tile([1, 32], F32, tag="u")
            nc.vector.tensor_reduce(
                out=u[:, 0:16],
                in_=tl[:, 0:128].rearrange("p (o g) -> p o g", g=G),
                axis=AX.X,
                op=ALU.min,
            )
            nc.vector.tensor_reduce(
                out=u[:, 16:32],
                in_=tl[:, 128:256].rearrange("p (o g) -> p o g", g=G),
                axis=AX.X,
                op=ALU.max,
            )
            # scale = 1/(mx-mn+eps); nbias = -mn*scale
            v = small.tile([1, 16], F32, tag="v")
            nc.vector.tensor_sub(out=v, in0=u[:, 16:32], in1=u[:, 0:16])
            vv = small.tile([1, 16], F32, tag="vv")
            nc.vector.tensor_scalar(
                out=vv, in0=v, scalar1=EPS, scalar2=None, op0=ALU.add
            )
            pk = small.tile([1, 32], F32, tag="pk")
            nc.vector.reciprocal(out=pk[:, 0:16], in_=vv)
            tmp = small.tile([1, 16], F32, tag="tmp")
            nc.vector.tensor_mul(out=tmp, in0=u[:, 0:16], in1=pk[:, 0:16])
            nc.vector.tensor_scalar(
                out=pk[:, 16:32], in0=tmp, scalar1=-1.0, scalar2=None, op0=ALU.mult
            )
            # broadcast [1,32] free-dim -> per-partition [128,2] via transpose
            # + selector matmuls (sel[k,0,p]=d(k==p//8), sel[k,1,p]=d(k-16==p//8))
            pz = psmall.tile([32, 1], F32, tag="psm")
            nc.tensor.transpose(pz, pk, id_sb[0:1, 0:1])
            zs = small.tile([32, 1], F32, tag="zs")
            nc.scalar.copy(out=zs, in_=pz)
            pb = psmall.tile([128, 2], F32, tag="psm")
            nc.tensor.matmul(pb[:, 0:1], sel_sb[:, 0, :], zs, start=True, stop=True)
            nc.tensor.matmul(pb[:, 1:2], sel_sb[:, 1, :], zs, start=True, stop=True)
            sc = small.tile([128, 2], F32, tag="sc", name=f"sc{bi}")
            nc.scalar.copy(out=sc, in_=pb)
            return sc

        def pass2_norm(bi, i, sc, eng):
            """Normalize resident bf16 y tile -> fp32 out tile on `eng`."""
            y_tiles = state[bi][2]
            ot = opool.tile([128, ts], F32, tag="o")
            if eng == "scalar":
                # Prelu with alpha=1 is identity: out = y*scale + nbias
                nc.scalar.activation(
                    out=ot,
                    in_=y_tiles[i],
                    func=AF.Prelu,
                    bias=sc[:, 1:2],
                    scale=sc[:, 0:1],
                    alpha=1.0,
                )
            else:
                e = nc.gpsimd if eng == "gpsimd" else nc.vector
                e.tensor_scalar(
                    out=ot,
                    in0=y_tiles[i],
                    scalar1=sc[:, 0:1],
                    scalar2=sc[:, 1:2],
                    op0=ALU.mult,
                    op1=ALU.add,
                )
            pending_kicks.append((bi, i, ot))

        def flush_kicks(keep=0):
            """Issue queued out-DMA kicks (oldest first) on the scalar queue."""
            while len(pending_kicks) > keep:
                bi, i, ot = pending_kicks.pop(0)
                nc.scalar.dma_start(
                    out=ys[bi, :, :, i * ts:(i + 1) * ts], in_=ot
                )

        for bi in range(BP):
            state[bi] = (
                spool.tile([128, ts], BF16, tag="rmin", name=f"rmin{bi}"),
                spool.tile([128, ts], BF16, tag="rmax", name=f"rmax{bi}"),
                [],
            )

        # head: pass 1 of batch 0, fold immediately (PE/Scalar have slack
        # to absorb the fold ops sitting ahead of batch-1 work)
        for i in range(n_t):
            pass1_tile(0, i)
        sc0 = stats_fold(0)
        # middle: pass 2 of batch 0 (GpSimd, otherwise idle) interleaved
        # with pass 1 of batch 1
        for i in range(n_t):
            pass1_tile(1, i)
            pass2_norm(0, i, sc0, "gpsimd")
            flush_kicks(keep=LAG)
        sc1 = stats_fold(1)
        # tail: pass 2 of batch 1, round-robin over the idle engines
        engs = ("vector", "gpsimd", "scalar")
        for i in range(n_t):
            pass2_norm(1, i, sc1, engs[i % 3])
            flush_kicks(keep=LAG)
        flush_kicks(keep=0)

    nc.compile()
    return nc


def host_consts(w, b):
    """Host-side tiny constant tensors fed to every core."""
    w = np.asarray(w, np.float32).reshape(16)
    b = np.asarray(b, np.float32).reshape(1)
    W2 = np.stack([np.roll(w, o) for o in range(16)], axis=0)   # [O,C]
    wbd = np.kron(W2.T.copy(), np.eye(G, dtype=np.float32))     # [128,128]
    wbd = np.ascontiguousarray(wbd, np.float32)
    ident = np.eye(128, dtype=np.float32)
    sel = np.zeros((32, 2, 128), np.float32)
    for p in range(128):
        sel[p // G, 0, p] = 1.0
        sel[16 + p // G, 1, p] = 1.0
    b128 = np.full((128, 1), float(b[0]), np.float32)
    return wbd, ident, sel, b128


_NC = None
LAST_RESULTS = None


def kernel(x, w, b):
    global _NC, LAST_RESULTS
    x = np.ascontiguousarray(np.asarray(x, np.float32))
    assert x.shape == (B, C, H, W)
    if _NC is None:
        _NC = build_nc()
    wbd, ident, sel, b128 = host_consts(w, b)

    xg = x.reshape(N_CORES, BP, C, G, S_FULL)
    in_maps = [
        {
            "x": np.ascontiguousarray(xg[ci]),
            "wbd": wbd,
            "ident": ident,
            "sel": sel,
            "b128": b128,
        }
        for ci in range(N_CORES)
    ]
    res = run_bass_kernel_spmd(_NC, in_maps, core_ids=list(range(N_CORES)))
    LAST_RESULTS = res
    out = np.concatenate([r["y"].reshape(BP, C, H, W) for r in res.results], axis=0)
    return out


# revision 19
# speedup vs baseline: 1.1763x; 1.0498x over previous
"""Trainium2 Bass kernel for ChannelCompression:
   y = minmax_norm_spatial(leaky_relu(circulant_1x1_conv(x) + b))

Sharding: pure data parallel over batch (16 batches -> 2 per core x 8 cores).

Per-core strategy (memory-roofline bound: read x once, write y once):
  - View each batch as [C=16, G=8, S=32768] and stack (c,g) onto the 128
    SBUF partitions.  The circulant 16x16 conv becomes one 128x128
    block-structured matmul weight kron(W2.T, I8), so every PE column
    computes all 16 output channels for 8 spatial groups at once.
  - The matmul runs in float32r (single-pass relaxed fp32, 1 cycle/row
    for moving >= 256), so PE is never the bottleneck.
  - Pass 1 streams x tiles in, matmuls into PSUM (fp32), applies
    leaky-relu (+bias) on ScalarE while copying PSUM -> resident SBUF y
    buffer in bf16 (8 MiB/batch -> both batches fit, fully
    double-buffered).  Spatial min/max runs on DVE as elementwise
    running-min/max accumulator tiles (bf16 tensor_tensor, 2x_1p fast
    mode) -- tensor_reduce has no fast mode, so reducing every tile
    would make DVE the critical path.  The last tile fuses the
    accumulate with the final reduction via tensor_tensor_reduce.
  - Per-batch stats are folded across the 8 spatial groups via tiny PE
    transposes into free-dim space, reduced, inverted, and broadcast
    back to per-partition scale/bias with two tiny selector matmuls.
  - Pass 2 normalizes the resident bf16 y back to fp32 and streams it
    out.  In the middle phase (overlapped with pass 1 of the next batch)
    GpSimd does the normalize; in the tail phase the work round-robins
    over GpSimd / Vector / Scalar.  Output DMA kicks are issued from the
    Scalar queue with a 1-tile lag so the kick's semaphore wait never
    blocks the next PReLU issue.
"""

import numpy as np
from contextlib import ExitStack

import concourse.bacc as bacc
import concourse.tile as tile
import concourse.bass as bass
from concourse import mybir
from concourse.bass_utils import run_bass_kernel_spmd

F32 = mybir.dt.float32
F32R = mybir.dt.float32r
BF16 = mybir.dt.bfloat16
AF = mybir.ActivationFunctionType
ALU = mybir.AluOpType
AX = mybir.AxisListType

N_CORES = 8
B, C, H, W = 16, 16, 512, 512
G = 8                   # spatial groups stacked into partitions
BP = B // N_CORES       # batches per core
S_FULL = (H * W) // G   # 32768 spatial elems per group
TS = 2048               # columns per tile
PT = 1024               # columns per PSUM tile (2 banks)
MM = 512                # columns per matmul (1 PSUM bank)
EPS = 1e-8
NEG_SLOPE = 0.1
LAG = 1                 # out-DMA kick lag (tiles) on the scalar queue
BIG = 3.0e38


def build_nc(S=S_FULL, ts=TS):
    n_t = S // ts
    nc = bacc.Bacc("TRN2", target_bir_lowering=False)

    xs = nc.dram_tensor("x", [BP, C, G, S], F32R, kind="ExternalInput")
    wbd = nc.dram_tensor("wbd", [128, 128], F32R, kind="ExternalInput")
    ident = nc.dram_tensor("ident", [128, 128], F32, kind="ExternalInput")
    sel = nc.dram_tensor("sel", [32, 2, 128], F32, kind="ExternalInput")
    bb = nc.dram_tensor("b128", [128, 1], F32, kind="ExternalInput")
    ys = nc.dram_tensor("y", [BP, C, G, S], F32, kind="ExternalOutput")

    with tile.TileContext(nc) as tc, ExitStack() as ctx:
        consts = ctx.enter_context(tc.tile_pool(name="consts", bufs=1))
        xpool = ctx.enter_context(tc.tile_pool(name="xpool", bufs=4))
        ypool = ctx.enter_context(tc.tile_pool(name="ypool", bufs=2 * n_t))
        opool = ctx.enter_context(tc.tile_pool(name="opool", bufs=3))
        spool = ctx.enter_context(tc.tile_pool(name="stats", bufs=2))
        small = ctx.enter_context(tc.tile_pool(name="small", bufs=2))
        psum = ctx.enter_context(tc.tile_pool(name="psum", bufs=3, space="PSUM"))
        psmall = ctx.enter_context(tc.tile_pool(name="psmall", bufs=2, space="PSUM"))

        wbd_sb = consts.tile([128, 128], F32R)
        nc.sync.dma_start(out=wbd_sb, in_=wbd[:])
        id_sb = consts.tile([128, 128], F32)
        nc.sync.dma_start(out=id_sb, in_=ident[:])
        sel_sb = consts.tile([32, 2, 128], F32)
        nc.sync.dma_start(out=sel_sb, in_=sel[:])
        b_sb = consts.tile([128, 1], F32)
        nc.sync.dma_start(out=b_sb, in_=bb[:])
        # warm the Prelu activation table during the DMA ramp so the first
        # real PReLU doesn't pay the ACT_TABLE_LOAD latency
        warm = consts.tile([128, 1], F32)
        nc.scalar.activation(
            out=warm, in_=b_sb, func=AF.Prelu, bias=b_sb, scale=1.0,
            alpha=NEG_SLOPE,
        )

        state = {}
        pending_kicks = []  # (bi, i, ot) awaiting a lagged scalar dma kick

        def pass1_tile(bi, i):
            """DMA in x tile i of batch bi, conv+lrelu into resident bf16 y."""
            rmin, rmax, y_tiles = state[bi]
            xt = xpool.tile([128, ts], F32R, tag="x")
            nc.sync.dma_start(out=xt, in_=xs[bi, :, :, i * ts:(i + 1) * ts])
            yt = ypool.tile([128, ts], BF16, tag="y")
            for j in range(ts // PT):
                pt = psum.tile([128, PT], F32, tag="ps")
                for k in range(PT // MM):
                    c0 = k * MM
                    nc.tensor.matmul(
                        pt[:, c0:c0 + MM],
                        wbd_sb,
                        xt[:, j * PT + c0:j * PT + c0 + MM],
                        start=True,
                        stop=True,
                    )
                # y = leaky_relu(conv + b): fused PSUM->SBUF (bf16) on ScalarE
                nc.scalar.activation(
                    out=yt[:, j * PT:(j + 1) * PT],
                    in_=pt,
                    func=AF.Prelu,
                    bias=b_sb,
                    scale=1.0,
                    alpha=NEG_SLOPE,
                )
            # running elementwise min/max accumulators on DVE (bf16 ops get
            # the 2x fast mode there; tensor_reduce per tile has none and
            # would make DVE the pacer)
            if i == 0:
                nc.vector.tensor_scalar(
                    out=rmin, in0=yt, scalar1=BIG, scalar2=None, op0=ALU.min
                )
                nc.vector.tensor_scalar(
                    out=rmax, in0=yt, scalar1=-BIG, scalar2=None, op0=ALU.max
                )
            else:
                nc.vector.tensor_tensor(out=rmin, in0=rmin, in1=yt, op=ALU.min)
                nc.vector.tensor_tensor(out=rmax, in0=rmax, in1=yt, op=ALU.max)
            y_tiles.append(yt)

        def stats_fold(bi):
            """Fold per-partition stats into per-partition scale/bias [128,2]."""
            rmin, rmax = state[bi][:2]
            # separate min/max stat tiles so each PE transpose can start as
            # soon as its own DVE reduce is done (overlapping the other)
            s2n = small.tile([128, 1], F32, tag="s2n")
            s2x = small.tile([128, 1], F32, tag="s2x")
            nc.vector.tensor_reduce(out=s2n, in_=rmin, axis=AX.X, op=ALU.min)
            ptr_min = psmall.tile([1, 128], F32, tag="psm")
            nc.tensor.transpose(ptr_min, s2n, id_sb)
            nc.vector.tensor_reduce(out=s2x, in_=rmax, axis=AX.X, op=ALU.max)
            ptr_max = psmall.tile([1, 128], F32, tag="psm")
            nc.tensor.transpose(ptr_max, s2x, id_sb)
            tl = small.tile([1, 256], F32, tag="tl")
            nc.scalar.copy(out=tl[:, 0:128], in_=ptr_min)
            nc.scalar.copy(out=tl[:, 128:256], in_=ptr_max)
            # reduce over the 8 groups (free index p = o*8+g)
            u = small.tile([1, 32], F32, tag="u")
            nc.vector.tensor_reduce(
                out=u[:, 0:16],
                in_=tl[:, 0:128].rearrange("p (o g) -> p o g", g=G),
                axis=AX.X,
                op=ALU.min,
            )
            nc.vector.tensor_reduce(
                out=u[:, 16:32],
                in_=tl[:, 128:256].rearrange("p (o g) -> p o g", g=G),
                axis=AX.X,
                op=ALU.max,
            )
            # scale = 1/(mx-mn+eps); nbias = -mn*scale
            v = small.tile([1, 16], F32, tag="v")
            nc.vector.tensor_sub(out=v, in0=u[:, 16:32], in1=u[:, 0:16])
            vv = small.tile([1, 16], F32, tag="vv")
            nc.vector.tensor_scalar(
                out=vv, in0=v, scalar1=EPS, scalar2=None, op0=ALU.add
            )
            pk = small.tile([1, 32], F32, tag="pk")
            nc.vector.reciprocal(out=pk[:, 0:16], in_=vv)
            tmp = small.tile([1, 16], F32, tag="tmp")
            nc.vector.tensor_mul(out=tmp, in0=u[:, 0:16], in1=pk[:, 0:16])
            nc.vector.tensor_scalar(
                out=pk[:, 16:32], in0=tmp, scalar1=-1.0, scalar2=None, op0=ALU.mult
            )
            # broadcast [1,32] free-dim -> per-partition [128,2] via transpose
            # + selector matmuls (sel[k,0,p]=d(k==p//8), sel[k,1,p]=d(k-16==p//8))
            pz = psmall.tile([32, 1], F32, tag="psm")
            nc.tensor.transpose(pz, pk, id_sb[0:1, 0:1])
            zs = small.tile([32, 1], F32, tag="zs")
            nc.scalar.copy(out=zs, in_=pz)
            pb = psmall.tile([128, 2], F32, tag="psm")
            nc.tensor.matmul(pb[:, 0:1], sel_sb[:, 0, :], zs, start=True, stop=True)
            nc.tensor.matmul(pb[:, 1:2], sel_sb[:, 1, :], zs, start=True, stop=True)
            sc = small.tile([128, 2], F32, tag="sc", name=f"sc{bi}")
            nc.scalar.copy(out=sc, in_=pb)
            return sc

        def pass2_norm(bi, i, sc, eng):
            """Normalize resident bf16 y tile -> fp32 out tile on `eng`."""
            y_tiles = state[bi][2]
            ot = opool.tile([128, ts], F32, tag="o")
            if eng == "scalar":
                # Prelu with alpha=1 is identity: out = y*scale + nbias
                nc.scalar.activation(
                    out=ot,
                    in_=y_tiles[i],
                    func=AF.Prelu,
                    bias=sc[:, 1:2],
                    scale=sc[:, 0:1],
                    alpha=1.0,
                )
            else:
                e = nc.gpsimd if eng == "gpsimd" else nc.vector
                e.tensor_scalar(
                    out=ot,
                    in0=y_tiles[i],
                    scalar1=sc[:, 0:1],
                    scalar2=sc[:, 1:2],
                    op0=ALU.mult,
                    op1=ALU.add,
                )
            pending_kicks.append((bi, i, ot))

        def flush_kicks(keep=0):
            """Issue queued out-DMA kicks (oldest first) on the scalar queue."""
            while len(pending_kicks) > keep:
                bi, i, ot = pending_kicks.pop(0)
                nc.scalar.dma_start(
                    out=ys[bi, :, :, i * ts:(i + 1) * ts], in_=ot
                )

        for bi in range(BP):
            state[bi] = (
                spool.tile([128, ts], BF16, tag="rmin", name=f"rmin{bi}"),
                spool.tile([128, ts], BF16, tag="rmax", name=f"rmax{bi}"),
                [],
            )

        # head: pass 1 of batch 0, fold immediately (PE/Scalar have slack
        # to absorb the fold ops sitting ahead of batch-1 work)
        for i in range(n_t):
            pass1_tile(0, i)
        sc0 = stats_fold(0)
        # middle: pass 2 of batch 0 (GpSimd, otherwise idle) interleaved
        # with pass 1 of batch 1
        for i in range(n_t):
            pass1_tile(1, i)
            pass2_norm(0, i, sc0, "gpsimd")
            flush_kicks(keep=LAG)
        sc1 = stats_fold(1)
        # tail: pass 2 of batch 1, round-robin over the idle engines
        engs = ("vector", "gpsimd", "scalar")
        for i in range(n_t):
            pass2_norm(1, i, sc1, engs[i % 3])
            flush_kicks(keep=LAG)
        flush_kicks(keep=0)

    nc.compile()
    return nc


def host_consts(w, b):
    """Host-side tiny constant tensors fed to every core."""
    w = np.asarray(w, np.float32).reshape(16)
    b = np.asarray(b, np.float32).reshape(1)
    W2 = np.stack([np.roll(w, o) for o in range(16)], axis=0)   # [O,C]
    wbd = np.kron(W2.T.copy(), np.eye(G, dtype=np.float32))     # [128,128]
    wbd = np.ascontiguousarray(wbd, np.float32)
    ident = np.eye(128, dtype=np.float32)
    sel = np.zeros((32, 2, 128), np.float32)
    for p in range(128):
        sel[p // G, 0, p] = 1.0
        sel[16 + p // G, 1, p] = 1.0
    b128 = np.full((128, 1), float(b[0]), np.float32)
    return wbd, ident, sel, b128


_NC = None
LAST_RESULTS = None


def kernel(x, w, b):
    global _NC, LAST_RESULTS
    x = np.ascontiguousarray(np.asarray(x, np.float32))
    assert x.shape == (B, C, H, W)
    if _NC is None:
        _NC = build_nc()
    wbd, ident, sel, b128 = host_consts(w, b)

    xg = x.reshape(N_CORES, BP, C, G, S_FULL)
    in_maps = [
        {
            "x": np.ascontiguousarray(xg[ci]),
            "wbd": wbd,
            "ident": ident,
            "sel": sel,
            "b128": b128,
        }
        for ci in range(N_CORES)
    ]
    res = run_bass_kernel_spmd(_NC, in_maps, core_ids=list(range(N_CORES)))
    LAST_RESULTS = res
    out = np.concatenate([r["y"].reshape(BP, C, H, W) for r in res.results], axis=0)
    return out
